# revision 1
# baseline (speedup 1.0000x reference)
"""Trainium2 Bass kernel for nn_InterpreMol_55877524521515.

6-layer post-norm transformer encoder, B=64 molecules, S=255(+CLS)=256,
D=512, H=8 heads, FF=2048, plus a 2-layer head on the CLS token.

Strategy: data-parallel over batch (8 molecules per NeuronCore, 8 cores).
Activations live in SBUF transposed ([D, seq]); residual stream f32r.
Each layer runs as two passes over all 8 molecules — attention(+LN1) then
FFN — with LN2 batched at the end of the FFN pass, so the Activation
engine's Exp and Gelu table sets swap only twice per layer. LN rstd uses
exp(-0.5*ln(var)) (ln+exp share a table set with attention's Exp).
Edge bias (+key-padding mask as -30000, fp16 [mol,head,k,q]) is added into
the score PSUM by an identity-matrix matmul on the PE, so softmax's Exp
reads PSUM directly. Softmax denominators come from a ones-augmented V;
per-query normalization = reciprocal (DVE) + partition_broadcast (Pool) +
multiply from PSUM (DVE). FFN weights and most intermediate tensors are
bf16; QKV/attention math stays f32r.
"""
import sys

sys.path.insert(0, "/opt/trn_rl_repo")

import numpy as np

import concourse.bass as bass
import concourse.tile as tile
from concourse import bacc, mybir
from concourse.bass import ds, ts
from concourse.bass_utils import run_bass_kernel_spmd

F32 = mybir.dt.float32
F32R = mybir.dt.float32r
F16 = mybir.dt.float16
F8 = mybir.dt.float8e4
BF16 = mybir.dt.bfloat16
AF = mybir.ActivationFunctionType
OP = mybir.AluOpType

B, S, D, H, L, FF, HID = 64, 255, 512, 8, 6, 2048, 256
S1 = 256          # seq with CLS
BL = 8            # molecules per core
DK = D // H       # 64
NCORE = 8
EPS = 1e-5
NEG = -30000.0    # masked-key bias (fp16-safe; exp underflows to 0 in f32)

# param packs (per layer, [128, c]), split so the attention-phase pack
# can reload while the FFN phase runs and vice versa:
#  ppa: 0:4 bq*0.125 | 4:8 bk | 8:12 bo | 12:16 ln1_g | 16:20 ln1_b
#  ppf: 0:16 b1 | 16:20 b2 | 20:24 ln2_g | 24:28 ln2_b
PPA = 20
PPF = 28

# HW bisect flags (default = full-featured kernel)
FLAGS = {"manual_tables": True, "pbcast": True, "fp8bias": True,
         "gps_ops": True}


def build_program(reps=1, unroll=False):
    nc = bacc.Bacc("TRN2", target_bir_lowering=False, debug=False)

    x0t_d = nc.dram_tensor("x0t", [D, BL * S1], F32R, kind="ExternalInput")
    bias_d = nc.dram_tensor("biast", [128, BL * H * 2 * S1], F8,
                            kind="ExternalInput")
    wq_d = nc.dram_tensor("wq", [L * D, D], F32R, kind="ExternalInput")
    wk_d = nc.dram_tensor("wk", [L * D, D], F32R, kind="ExternalInput")
    wv_d = nc.dram_tensor("wv", [L * D, D], F32R, kind="ExternalInput")
    wo_d = nc.dram_tensor("wo", [L * D, D], F16, kind="ExternalInput")
    w1_d = nc.dram_tensor("w1", [L * D, FF], F16, kind="ExternalInput")
    w2_d = nc.dram_tensor("w2", [L * FF, D], F16, kind="ExternalInput")
    ppa_d = nc.dram_tensor("ppa", [L * 128, PPA], F32, kind="ExternalInput")
    ppf_d = nc.dram_tensor("ppf", [L * 128, PPF], F32, kind="ExternalInput")
    pv_d = nc.dram_tensor("pv", [L, D], F32R, kind="ExternalInput")
    eye_d = nc.dram_tensor("eye", [128, 128], F8, kind="ExternalInput")
    sel_d = nc.dram_tensor("sel", [8, 512], F32R, kind="ExternalInput")
    sel2_d = nc.dram_tensor("sel2", [1, 64], F16, kind="ExternalInput")
    hw1_d = nc.dram_tensor("hw1", [D, HID], F32R, kind="ExternalInput")
    hb1_d = nc.dram_tensor("hb1", [128, 2], F32, kind="ExternalInput")
    hw2_d = nc.dram_tensor("hw2", [128, 2], F32R, kind="ExternalInput")
    hb2_d = nc.dram_tensor("hb2", [1, 1], F32, kind="ExternalInput")
    out_d = nc.dram_tensor("out", [1, BL], F32, kind="ExternalOutput")

    with tile.TileContext(nc) as tc:
        with tc.tile_pool(name="cst", bufs=1) as cst, \
             tc.tile_pool(name="qtp", bufs=8) as qtp, \
             tc.tile_pool(name="ktp", bufs=8) as ktp, \
             tc.tile_pool(name="vgp", bufs=4) as vgp, \
             tc.tile_pool(name="exp_", bufs=3) as exp_, \
             tc.tile_pool(name="atp", bufs=8) as atp, \
             tc.tile_pool(name="xap", bufs=4) as xap, \
             tc.tile_pool(name="htp", bufs=17) as htp, \
             tc.tile_pool(name="sqp", bufs=2) as sqp, \
             tc.tile_pool(name="lnp", bufs=6) as lnp, \
             tc.tile_pool(name="rcp", bufs=2) as rcp, \
             tc.tile_pool(name="dnp", bufs=3) as dnp, \
             tc.tile_pool(name="bcp", bufs=6) as bcp, \
             tc.tile_pool(name="psq", bufs=2, space="PSUM") as psq, \
             tc.tile_pool(name="pso", bufs=1, space="PSUM") as pso, \
             tc.tile_pool(name="psw", bufs=1, space="PSUM") as psw, \
             tc.tile_pool(name="pss", bufs=2, space="PSUM") as pss, \
             tc.tile_pool(name="psa", bufs=2, space="PSUM") as psa:

            # ---- static tiles -------------------------------------------
            xres = [[cst.tile([128, S1], F32R, name=f"xres_{kt}_{m}")
                     for m in range(BL)] for kt in range(4)]
            xl = [[cst.tile([128, S1], F16, name=f"xl_{kt}_{m}")
                   for m in range(BL)] for kt in range(4)]
            wq_sb = [cst.tile([128, D], F32R, name=f"wq_sb{kt}") for kt in range(4)]
            wk_sb = [cst.tile([128, D], F32R, name=f"wk_sb{kt}") for kt in range(4)]
            wv_sb = [cst.tile([128, D], F32R, name=f"wv_sb{kt}") for kt in range(4)]
            wo_sb = [cst.tile([128, D], F16, name=f"wo_sb{kt}") for kt in range(4)]
            w1_sb = [cst.tile([128, FF], F16, name=f"w1_sb{kt}") for kt in range(4)]
            w2_sb = [cst.tile([128, D], F16, name=f"w2_sb{kt}") for kt in range(16)]
            ppa_sb = cst.tile([128, PPA], F32, name="ppa_sb")
            ppf_sb = cst.tile([128, PPF], F32, name="ppf_sb")
            bias_sb = cst.tile([128, BL, H, 2, S1], F8, name="bias_sb")
            pv_sb = cst.tile([1, D], F32R, name="pv_sb")
            eye_sb = cst.tile([128, 128], F8, name="eye_sb")
            sel_sb = cst.tile([8, 512], F32R, name="sel_sb")
            sel2_sb = cst.tile([1, 64], F16, name="sel2_sb")
            ones_inv = cst.tile([128, 128], F32R, name="ones_inv")   # 1/512
            ones_ib = cst.tile([128, 128], F16, name="ones_ib")     # 1/512
            ones_r = cst.tile([1, 128], F32R, name="ones_r")         # 1.0
            ones8 = cst.tile([128, 8], F16, name="ones8")           # 1.0
            hw1_sb = [cst.tile([128, HID], F32R, name=f"hw1_sb{kt}") for kt in range(4)]
            hb1_sb = cst.tile([128, 2], F32, name="hb1_sb")
            hw2_sb = cst.tile([128, 2], F32R, name="hw2_sb")
            hb2_sb = cst.tile([1, 1], F32, name="hb2_sb")
            cls_sb = [cst.tile([128, BL], F32R, name=f"cls_sb{kt}") for kt in range(4)]
            h_sb = [cst.tile([128, BL], F32R, name=f"h_sb{mt}") for mt in range(2)]
            out_sb = cst.tile([1, BL], F32, name="out_sb")
            cinit = cst.tile([128, 128], F32, name="cinit")

            # ---- constants ----------------------------------------------
            nc.vector.memset(cinit[:], 1.0 / D)
            nc.vector.tensor_copy(ones_inv[:], cinit[:])
            nc.vector.tensor_copy(ones_ib[:], cinit[:])
            nc.vector.memset(cinit[:], 1.0)
            nc.vector.tensor_copy(ones_r[:], cinit[0:1, :])
            nc.vector.tensor_copy(ones8[:], cinit[:, 0:8])

            # ---- initial loads ------------------------------------------
            for kt in range(4):
                for m in range(BL):
                    nc.sync.dma_start(
                        out=xres[kt][m][:],
                        in_=x0t_d.ap()[kt * 128:(kt + 1) * 128, m * S1:(m + 1) * S1])
            nc.sync.dma_start(out=eye_sb[:], in_=eye_d.ap())
            nc.sync.dma_start(out=sel_sb[:], in_=sel_d.ap())
            nc.sync.dma_start(out=sel2_sb[:], in_=sel2_d.ap())
            bstr = H * 2 * S1
            for m in range(BL):
                eng = (nc.sync, nc.scalar)[m % 2]
                eng.dma_start(
                    out=bias_sb[:, m],
                    in_=bias_d.ap()[:, m * bstr:(m + 1) * bstr].rearrange(
                        "p (h a q) -> p h a q", h=H, a=2))
            for kt in range(4):
                nc.sync.dma_start(out=hw1_sb[kt][:],
                                  in_=hw1_d.ap()[kt * 128:(kt + 1) * 128, :])
            nc.sync.dma_start(out=hb1_sb[:], in_=hb1_d.ap())
            nc.sync.dma_start(out=hw2_sb[:], in_=hw2_d.ap())
            nc.sync.dma_start(out=hb2_sb[:], in_=hb2_d.ap())

            def load_weights(iv):
                for kt in range(4):
                    nc.sync.dma_start(out=wq_sb[kt][:],
                                      in_=wq_d.ap()[ds(iv * D + kt * 128, 128), :])
                for kt in range(4):
                    nc.sync.dma_start(out=wk_sb[kt][:],
                                      in_=wk_d.ap()[ds(iv * D + kt * 128, 128), :])
                for kt in range(4):
                    nc.sync.dma_start(out=wv_sb[kt][:],
                                      in_=wv_d.ap()[ds(iv * D + kt * 128, 128), :])
                nc.sync.dma_start(out=ppa_sb[:], in_=ppa_d.ap()[ds(iv * 128, 128), :])
                nc.sync.dma_start(out=ppf_sb[:], in_=ppf_d.ap()[ds(iv * 128, 128), :])
                nc.sync.dma_start(out=pv_sb[:], in_=pv_d.ap()[ds(iv, 1), :])
                for kt in range(4):
                    nc.gpsimd.dma_start(out=wo_sb[kt][:],
                                        in_=wo_d.ap()[ds(iv * D + kt * 128, 128), :])

            # w1/w2 DMA issue is deferred into the attention pass so the
            # Pool engine's SWDGE descriptor generation doesn't pile up at
            # the layer boundary where LN2/broadcast work needs Pool.
            def load_w1(iv):
                for kt in range(4):
                    nc.gpsimd.dma_start(out=w1_sb[kt][:],
                                        in_=w1_d.ap()[ds(iv * D + kt * 128, 128), :])

            def load_w2(iv, lo, hi):
                for kt in range(lo, hi):
                    nc.gpsimd.dma_start(out=w2_sb[kt][:],
                                        in_=w2_d.ap()[ds(iv * FF + kt * 128, 128), :])

            # layer-norm over the partition (D) dim of 4 x [128, S1] tiles;
            # rstd = exp(-0.5*ln(var)) keeps the Act engine in the exp table
            # set. dst_f(kt) returns the destination AP for tile kt.
            def layer_norm(x_t, pp, gcol, dst_f, sq_eng="act",
                           fin_eng="pool", bf=True):
                oi = ones_ib if bf else ones_inv
                sdt = F16 if bf else F32R
                # mean and E[x^2] as two 1-bank tiles in the av pool (the
                # v/sc/qk rings stay decoupled from LN). Only ln+exp (rstd)
                # plus optionally Square touch the Act engine.
                ps_mn = psa.tile([128, S1], F32, name="ps_mn", tag="av")
                for kt in range(4):
                    nc.tensor.matmul(ps_mn[:], oi[:], x_t[kt],
                                     start=(kt == 0), stop=(kt == 3))
                ps_sq = psa.tile([128, S1], F32, name="ps_sq", tag="av")
                for kt in range(4):
                    sq = sqp.tile([128, S1], sdt, name="sq")
                    if sq_eng == "act":
                        nc.scalar.activation(sq[:], x_t[kt], AF.Square)
                    elif sq_eng == "pool" and FLAGS["gps_ops"]:
                        nc.gpsimd.tensor_mul(sq[:], x_t[kt], x_t[kt])
                    else:
                        nc.vector.tensor_mul(sq[:], x_t[kt], x_t[kt])
                    nc.tensor.matmul(ps_sq[:], oi[:], sq[:],
                                     start=(kt == 0), stop=(kt == 3))
                mean = lnp.tile([128, S1], F32, name="mean", tag="ln")
                nc.vector.tensor_copy(mean[:], ps_mn[:])
                m2 = lnp.tile([128, S1], F32, name="m2", tag="ln")
                nc.vector.tensor_mul(m2[:], mean[:], mean[:])
                var = lnp.tile([128, S1], F32, name="var", tag="ln")
                nc.vector.scalar_tensor_tensor(var[:], ps_sq[:], EPS,
                                               m2[:], op0=OP.add,
                                               op1=OP.subtract)
                lnv = lnp.tile([128, S1], F32, name="lnv", tag="ln")
                nc.scalar.activation(lnv[:], var[:], AF.Ln)
                rstd = lnp.tile([128, S1], F32, name="rstd", tag="ln")
                nc.scalar.activation(rstd[:], lnv[:], AF.Exp, scale=-0.5)
                for kt in range(4):
                    cen = lnp.tile([128, S1], F32, name="cen", tag="ln")
                    nc.vector.tensor_sub(cen[:], x_t[kt], mean[:])
                    nrm = lnp.tile([128, S1], F32, name="nrm", tag="ln")
                    if FLAGS["gps_ops"]:
                        nc.gpsimd.tensor_mul(nrm[:], cen[:], rstd[:])
                    else:
                        nc.vector.tensor_mul(nrm[:], cen[:], rstd[:])
                    g = pp[:, gcol + kt:gcol + 1 + kt]
                    b = pp[:, gcol + 4 + kt:gcol + 5 + kt]
                    if fin_eng == "pool" and FLAGS["gps_ops"]:
                        nc.gpsimd.tensor_scalar(dst_f(kt), nrm[:], g, b,
                                                op0=OP.mult, op1=OP.add)
                    else:
                        nc.vector.tensor_scalar(dst_f(kt), nrm[:], g, b,
                                                op0=OP.mult, op1=OP.add)

            # exp+ln share one table set ('natural_log_exp_and_others',
            # which also has square/identity/copy) and gelu another; the
            # auto-insert pass picks per-function canonical sets (exp->0,
            # ln->5) and would thrash, so place the loads explicitly at
            # phase edges and skip the auto pass (all Act funcs used here —
            # Square/Ln/Exp/Identity/Gelu — are covered on every path).
            from concourse.hw_specs import get_activation_tables
            _tbl_names = list(get_activation_tables(nc.m.arch).keys())
            SET_EXP = _tbl_names.index("natural_log_exp_and_others")
            SET_GELU = _tbl_names.index("gelu_and_others")

            def load_act_set(set_id):
                # table placement is handled by the (reordered) auto pass
                return None

            def layer_body(iv):
                load_weights(iv)
                # ================= attention pass ========================
                load_act_set(SET_EXP)
                for m in range(BL):
                    # ---- Q^T, K^T projections ([dout,128] x [din,seq]) --
                    qt_t = []
                    kt_t = []
                    for mt in range(4):
                        ps_qk = psq.tile([128, 2, S1], F32, name="ps_qk",
                                         tag="q2")
                        for kt in range(4):
                            nc.tensor.matmul(
                                ps_qk[:, 0, :],
                                wq_sb[kt][:, mt * 128:(mt + 1) * 128],
                                xres[kt][m][:],
                                start=(kt == 0), stop=(kt == 3))
                        for kt in range(4):
                            nc.tensor.matmul(
                                ps_qk[:, 1, :],
                                wk_sb[kt][:, mt * 128:(mt + 1) * 128],
                                xres[kt][m][:],
                                start=(kt == 0), stop=(kt == 3))
                        q = qtp.tile([128, S1], F16, name="q")
                        nc.scalar.activation(q[:], ps_qk[:, 0, :], AF.Identity,
                                             bias=ppa_sb[:, mt:mt + 1])
                        qt_t.append(q)
                        k = ktp.tile([128, S1], F16, name="k")
                        nc.scalar.activation(k[:], ps_qk[:, 1, :], AF.Identity,
                                             bias=ppa_sb[:, 4 + mt:5 + mt])
                        kt_t.append(k)

                    # ---- V natural ([seq,512]) + ones column ------------
                    vg_t = []
                    for st in range(2):
                        ps_v = psw.tile([128, 512], F32, name="ps_v", tag="w")
                        for kt in range(4):
                            nc.tensor.matmul(
                                ps_v[:],
                                xres[kt][m][:, st * 128:(st + 1) * 128],
                                wv_sb[kt][:],
                                start=(kt == 0), stop=False)
                        nc.tensor.matmul(ps_v[:], ones_r[:], pv_sb[:],
                                         start=False, stop=True)
                        vg = vgp.tile([128, H, DK + 1], F16, name="vg")
                        nc.vector.tensor_copy(
                            vg[:, :, 0:DK],
                            ps_v[:].rearrange("p (h d) -> p h d", h=H))
                        nc.vector.tensor_copy(
                            vg[:, :, DK:DK + 1],
                            ones8[:].rearrange("p (h o) -> p h o", o=1))
                        vg_t.append(vg)

                    # ---- attention, heads software-pipelined ------------
                    # scores(h+1) are emitted before attn@V(h) so the PE's
                    # in-order queue does useful work while exp(h) runs on
                    # the Act engine.
                    at_t = [atp.tile([128, S1], F16, name="at") for _ in range(4)]
                    rc8 = rcp.tile([8, S1], F32R, name="rc8", tag="rc")

                    def emit_scores(h):
                        r0 = (h % 2) * 64
                        ps_sc = pss.tile([128, 2, S1], F32, name="ps_sc",
                                         tag="s")
                        ex = exp_.tile([128, 2, S1], F16, name="ex")
                        for st in range(2):
                            if FLAGS["fp8bias"]:
                                nc.tensor.matmul(
                                    ps_sc[:, st, :],
                                    eye_sb[:],
                                    bias_sb[:, m, h, st, :],
                                    start=True, stop=False)
                            nc.tensor.matmul(
                                ps_sc[:, st, :],
                                kt_t[h // 2][r0:r0 + 64, st * 128:(st + 1) * 128],
                                qt_t[h // 2][r0:r0 + 64, :],
                                start=not FLAGS["fp8bias"], stop=True)
                            nc.scalar.activation(ex[:, st, :],
                                                 ps_sc[:, st, :], AF.Exp)
                        return ex

                    den_ps = pso.tile([128, 2, S1], F32, name="den_ps",
                                      tag="o")
                    ex_cur = emit_scores(0)
                    for h in range(8):
                        r0 = (h % 2) * 64
                        ex = ex_cur
                        if h < 7:
                            ex_cur = emit_scores(h + 1)
                        ps_av = psa.tile([128, S1], F32, name="ps_av", tag="av")
                        for st in range(2):
                            nc.tensor.matmul(
                                ps_av[0:DK + 1, :],
                                vg_t[st][:, h, :],
                                ex[:, st, :],
                                start=(st == 0), stop=(st == 1))
                        den = dnp.tile([1, S1], F16, name="den")
                        nc.scalar.activation(den[0:1, :],
                                             ps_av[DK:DK + 1, :], AF.Identity)
                        nc.scalar.activation(at_t[h // 2][r0:r0 + 64, :],
                                             ps_av[0:DK, :], AF.Identity)
                        # gather this head's denominator row onto partition h
                        nc.tensor.matmul(
                            den_ps[0:8, 0, :],
                            sel2_sb[0:1, h * 8:(h + 1) * 8],
                            den[0:1, :],
                            start=(h == 0), stop=(h == 7))
                    # batch-normalize: one reciprocal over all heads, then a
                    # selection matmul broadcasts each head row into
                    # [128,S1] pair tiles, and in-place bf16 muls (DVE 2x)
                    with nc.allow_low_precision(reason="softmax recip"):
                        nc.vector.reciprocal(rc8[:], den_ps[0:8, 0, :])
                    for kt in range(4):
                        ps_bc = psa.tile([128, S1], F32, name="ps_bc",
                                         tag="av")
                        nc.tensor.matmul(
                            ps_bc[:], sel_sb[:, kt * 128:(kt + 1) * 128],
                            rc8[:], start=True, stop=True)
                        bc = bcp.tile([128, S1], F16, name="bc")
                        nc.scalar.activation(bc[:], ps_bc[:], AF.Identity)
                        nc.vector.tensor_mul(at_t[kt][:], at_t[kt][:], bc[:])

                    # ---- out proj + residual + LN1 ----------------------
                    xa_t = []
                    for mp in range(2):
                        ps_o = pso.tile([128, 2, S1], F32, name="ps_o",
                                        tag="o")
                        for half in range(2):
                            mt = 2 * mp + half
                            for kt in range(4):
                                nc.tensor.matmul(
                                    ps_o[:, half, :],
                                    wo_sb[kt][:, mt * 128:(mt + 1) * 128],
                                    at_t[kt][:],
                                    start=(kt == 0), stop=(kt == 3))
                        for half in range(2):
                            mt = 2 * mp + half
                            xa = xap.tile([128, S1], F16, name="xa")
                            nc.vector.scalar_tensor_tensor(
                                xa[:], ps_o[:, half, :],
                                ppa_sb[:, 8 + mt:9 + mt],
                                xres[mt][m][:], op0=OP.add, op1=OP.add)
                            xa_t.append(xa)

                    layer_norm([xa_t[kt][:] for kt in range(4)], ppa_sb,
                               12, lambda kt: xl[kt][m][:])
                    if m == 0:
                        load_w1(iv)
                    elif m == 1:
                        load_w2(iv, 0, 8)
                    elif m == 2:
                        load_w2(iv, 8, 16)

                # ================= FFN pass ==============================
                load_act_set(SET_GELU)
                for m in range(BL):
                    ht_t = []
                    for fp in range(8):
                        ps_f = psq.tile([128, 2, S1], F32, name="ps_f",
                                        tag="q2")
                        for half in range(2):
                            fb = 2 * fp + half
                            for kt in range(4):
                                nc.tensor.matmul(
                                    ps_f[:, half, :],
                                    w1_sb[kt][:, fb * 128:(fb + 1) * 128],
                                    xl[kt][m][:],
                                    start=(kt == 0), stop=(kt == 3))
                        for half in range(2):
                            fb = 2 * fp + half
                            ht = htp.tile([128, S1], F16, name="ht")
                            nc.scalar.activation(
                                ht[:], ps_f[:, half, :], AF.Gelu,
                                bias=ppf_sb[:, fb:fb + 1])
                            ht_t.append(ht)

                    for mp in range(2):
                        ps_g = psq.tile([128, 2, S1], F32, name="ps_g",
                                        tag="q2")
                        for half in range(2):
                            mt = 2 * mp + half
                            for kt in range(16):
                                nc.tensor.matmul(
                                    ps_g[:, half, :],
                                    w2_sb[kt][:, mt * 128:(mt + 1) * 128],
                                    ht_t[kt][:],
                                    start=(kt == 0), stop=(kt == 15))
                        # xres <- pre-LN2 value (normalized in batch below)
                        for half in range(2):
                            mt = 2 * mp + half
                            nc.vector.scalar_tensor_tensor(
                                xres[mt][m][:], ps_g[:, half, :],
                                ppf_sb[:, 16 + mt:17 + mt],
                                xl[mt][m][:], op0=OP.add, op1=OP.add)

                    # LN2 in two half-batches woven into the FFN pass (one
                    # exp<->gelu table round-trip each) so molecules 0-3 are
                    # ready for the next layer's attention early and the Act
                    # engine isn't a serial wall at the layer boundary.
                    if m == 3 or m == 7:
                        load_act_set(SET_EXP)
                        for mb in range(m - 3, m + 1):
                            layer_norm([xres[kt][mb][:] for kt in range(4)],
                                       ppf_sb, 20,
                                       lambda kt, mb=mb: xres[kt][mb][:],
                                       sq_eng="dve", fin_eng="dve",
                                       bf=False)
                        if m == 3:
                            load_act_set(SET_GELU)

            # layers are fully unrolled (static weight-tile ping-pong and
            # cross-layer overlap); reps>1 wraps the unrolled body in a
            # hardware loop for on-device repeat timing.
            if reps > 1:
                with tc.For_i(0, reps, 1) as rv:
                    for iv in range(L):
                        layer_body(iv)
            else:
                for iv in range(L):
                    layer_body(iv)

            # ---- head on CLS tokens -------------------------------------
            load_act_set(SET_GELU)
            for kt in range(4):
                for m in range(BL):
                    nc.vector.tensor_copy(cls_sb[kt][:, m:m + 1],
                                          xres[kt][m][:, 0:1])
            ps_h = psq.tile([128, 2, S1], F32, name="ps_h", tag="q2")
            for mt in range(2):
                for kt in range(4):
                    nc.tensor.matmul(
                        ps_h[:, mt, 0:BL],
                        hw1_sb[kt][:, mt * 128:(mt + 1) * 128],
                        cls_sb[kt][:],
                        start=(kt == 0), stop=(kt == 3))
            for mt in range(2):
                nc.scalar.activation(h_sb[mt][:], ps_h[:, mt, 0:BL], AF.Gelu,
                                     bias=hb1_sb[:, mt:mt + 1])
            ps_out = psq.tile([128, 2, S1], F32, name="ps_out", tag="q2")
            for mt in range(2):
                nc.tensor.matmul(ps_out[0:1, 0, 0:BL], hw2_sb[:, mt:mt + 1],
                                 h_sb[mt][:], start=(mt == 0), stop=(mt == 1))
            nc.scalar.activation(out_sb[:], ps_out[0:1, 0, 0:BL], AF.Identity,
                                 bias=hb2_sb[0:1, 0:1])
            nc.sync.dma_start(out=out_d.ap(), in_=out_sb[:])

    if FLAGS["manual_tables"]:
        # Run the auto table-load pass with a reordered table list so its
        # per-function canonical set for ln and exp is the shared
        # 'natural_log_exp_and_others' set (no ln<->exp thrash), then remap
        # the emitted ids back to true act_info.json indices for walrus.
        from concourse.hw_specs import get_activation_tables
        import bass_rust as _br

        def _patched_tables():
            tabs = get_activation_tables(nc.m.arch)
            names = list(tabs.keys())
            pref = ["natural_log_exp_and_others", "gelu_and_others"]
            order = pref + [n for n in names if n not in pref]
            _br.insert_act_table_loads(nc, [(n, tabs[n]) for n in order])
            remap = {i: names.index(n) for i, n in enumerate(order)}
            for b in nc.main_func.blocks:
                for inst in b.instructions:
                    if isinstance(inst, mybir.InstLoadActFuncSet):
                        inst.act_func_set_id = remap[inst.act_func_set_id]

        nc.insert_act_table_loads = _patched_tables
    nc.compile()
    return nc


_CACHE = {}


def _get_program(reps):
    if reps not in _CACHE:
        _CACHE[reps] = build_program(reps)
    return _CACHE[reps]


def prep_inputs(atom_emb, edge_bias, key_padding_mask, cls_token, Wq, bq, Wk,
                bk, Wv, bv, Wo, bo, ln1_g, ln1_b, W1, b1, W2, b2, ln2_g,
                ln2_b, head_W1, head_b1, head_W2, head_b2):
    import ml_dtypes
    f32 = np.float32
    atom_emb = np.asarray(atom_emb, f32)
    cls_token = np.asarray(cls_token, f32)
    x0 = np.concatenate(
        [np.broadcast_to(cls_token, (B, 1, D)), atom_emb], axis=1)  # [B,S1,D]

    # biasT[b,h,k,q] = edge_bias[b,q-1,k-1,h], scaled by 16 and stored in
    # fp8 e4m3; the on-device identity matmul uses eye=1/16 to undo the
    # scale. Masked key rows -> -240 (fp8 min) => -15 after descale, which
    # exp() makes negligible. Layout [p(k%128), b, h, a(k//128), q].
    f8 = ml_dtypes.float8_e4m3
    bt = np.zeros((B, H, S1, S1), f32)
    eb = np.asarray(edge_bias, f32).transpose(0, 3, 2, 1)  # [b,h,j(k),i(q)]
    bt[:, :, 1:, 1:] = eb * 16.0
    km = np.asarray(key_padding_mask, bool)
    bi, ki = np.nonzero(km)
    bt[bi, :, ki + 1, :] = -240.0
    bt8 = np.ascontiguousarray(
        bt.reshape(B, H, 2, 128, S1).transpose(3, 0, 1, 2, 4)).astype(f8)
    # bt8: [128, B, H, 2, S1]

    def seg(x):  # [L, dim] -> [L, dim//128, 128] -> [L, 128, dim//128]
        x = np.asarray(x, f32)
        return x.reshape(L, -1, 128).transpose(0, 2, 1)

    ppa = np.zeros((L, 128, PPA), f32)
    ppa[:, :, 0:4] = seg(np.asarray(bq, f32) * 0.125)
    ppa[:, :, 4:8] = seg(bk)
    ppa[:, :, 8:12] = seg(bo)
    ppa[:, :, 12:16] = seg(ln1_g)
    ppa[:, :, 16:20] = seg(ln1_b)
    ppf = np.zeros((L, 128, PPF), f32)
    ppf[:, :, 0:16] = seg(b1)
    ppf[:, :, 16:20] = seg(b2)
    ppf[:, :, 20:24] = seg(ln2_g)
    ppf[:, :, 24:28] = seg(ln2_b)

    shared = {
        "wq": np.ascontiguousarray(
            (np.asarray(Wq, f32) * 0.125).reshape(L * D, D)),
        "wk": np.ascontiguousarray(np.asarray(Wk, f32).reshape(L * D, D)),
        "wv": np.ascontiguousarray(np.asarray(Wv, f32).reshape(L * D, D)),
        "wo": np.ascontiguousarray(
            np.asarray(Wo, f32).reshape(L * D, D).astype(np.float16)),
        "w1": np.ascontiguousarray(
            np.asarray(W1, f32).reshape(L * D, FF).astype(np.float16)),
        "w2": np.ascontiguousarray(
            np.asarray(W2, f32).reshape(L * FF, D).astype(np.float16)),
        "ppa": np.ascontiguousarray(ppa.reshape(L * 128, PPA)),
        "ppf": np.ascontiguousarray(ppf.reshape(L * 128, PPF)),
        "pv": np.ascontiguousarray(np.asarray(bv, f32)),
        "eye": (np.eye(128, dtype=f32) / 16.0).astype(f8),
        "sel": np.ascontiguousarray(
            np.repeat(np.eye(8, dtype=f32), 64, axis=1)),
        "sel2": np.ascontiguousarray(
            np.eye(8, dtype=f32).reshape(1, 64)).astype(np.float16),
        "hw1": np.ascontiguousarray(np.asarray(head_W1, f32)),
        "hb1": np.ascontiguousarray(
            np.asarray(head_b1, f32).reshape(2, 128).T),
        "hw2": np.ascontiguousarray(
            np.asarray(head_W2, f32).reshape(2, 128).T),
        "hb2": np.asarray(head_b2, f32).reshape(1, 1),
    }
    in_maps = []
    for c in range(NCORE):
        sl = slice(c * BL, (c + 1) * BL)
        x0t = np.ascontiguousarray(
            x0[sl].transpose(2, 0, 1).reshape(D, BL * S1))
        in_maps.append({
            "x0t": x0t,
            "biast": np.ascontiguousarray(
                bt8[:, sl].reshape(128, BL * H * 2 * S1)),
            **shared})
    return in_maps


def run(in_maps, reps=1):
    nc = _get_program(reps)
    res = run_bass_kernel_spmd(nc, in_maps, list(range(NCORE)))
    out = np.concatenate([res.results[c]["out"].reshape(BL, 1)
                          for c in range(NCORE)], axis=0)
    return out


def kernel(**inputs) -> np.ndarray:
    in_maps = prep_inputs(**inputs)
    return run(in_maps, reps=1)

